# revision 9
# baseline (speedup 1.0000x reference)
"""Bahdanau attention (with coverage) Trainium2 Bass kernel.

Problem (per full input):
  query    [32, 1024]      f32
  values   [32, 2048, 1024] f32
  coverage [32, 2048, 1]   f32
  W1 [1024,1024], b1[1024]  (values proj)
  W2 [1024,1024], b2[1024]  (query proj)
  W3 [1,1024],   b3[1024]   (coverage proj)
  V  [1024,1],   bV[1]      (score proj; bV cancels in softmax)

  score   = tanh(values@W1 + b1 + (query@W2 + b2)[:,None,:] + coverage@W3 + b3) @ V + bV
  att     = softmax(score, axis=1)
  new_cov = coverage + att
  context = sum_s att * values

Sharding: data-parallel over batch across 8 cores (4 batches/core),
weights replicated. No cross-core communication.

Per-core schedule (per batch b, fully unrolled; engines run ahead of PE
in their in-order queues, which gives cross-batch prefetch for free):
  Pool  covb + 16 values tiles as casting loads f32->bf16 (SWDGE), then
        att_col(b-1) readback + 16 bf16 re-read tiles for context(b-1).
  SP    one 3D-out xbar transpose per values tile ([128,1024]bf16 ->
        [128,8,128] d-major) into VT(b); deferred output DMAs of b-1.
  PE    32x [8 GEMM1 mms + 1 K=1 coverage-fold mm] per batch; score mms
        deferred one slot so PE never waits on ACT tanh; context(b-1)
        mms interleaved at slots 8..23.
  ACT   tanh(psum + bias[u]) -> tT bf16 (bias = q_proj+b1+b2+b3), exp.
  DVE   score psum->row copies, softmax (max/recip/scale), new_cov add.
"""

import sys

if "/opt/trn_rl_repo" not in sys.path:
    sys.path.insert(0, "/opt/trn_rl_repo")

import numpy as np

import concourse.bass as bass
import concourse.mybir as mybir
import concourse.tile as tile
from concourse import bacc
from concourse.bass_utils import run_bass_kernel_spmd

B, S, D, U = 32, 2048, 1024, 1024
NCORES = 8
BL = B // NCORES  # 4 batches per core
P = 128
DJ = D // P   # 8 d-chunks
UK = U // P   # 8 u-chunks
SC = 512      # matmul moving free size
NSC = S // SC  # 4 s-chunks
NST = S // P   # 16 s-tiles
F32 = mybir.dt.float32
BF16 = mybir.dt.bfloat16
TANH = mybir.ActivationFunctionType.Tanh
EXP = mybir.ActivationFunctionType.Exp

_CACHE = {}


def _build():
    nc = bacc.Bacc("TRN2", target_bir_lowering=False, debug=False)

    q_d = nc.dram_tensor("q", [BL, D], F32, kind="ExternalInput")
    v_d = nc.dram_tensor("v", [BL, S, D], F32, kind="ExternalInput")
    cov_d = nc.dram_tensor("cov", [BL, S], F32, kind="ExternalInput")
    w1_d = nc.dram_tensor("w1", [D, U], F32, kind="ExternalInput")
    w2_d = nc.dram_tensor("w2", [D, U], F32, kind="ExternalInput")
    w3_d = nc.dram_tensor("w3", [U], F32, kind="ExternalInput")
    vv_d = nc.dram_tensor("vv", [U], F32, kind="ExternalInput")
    b1_d = nc.dram_tensor("b1", [U], F32, kind="ExternalInput")
    b2_d = nc.dram_tensor("b2", [U], F32, kind="ExternalInput")
    b3_d = nc.dram_tensor("b3", [U], F32, kind="ExternalInput")

    ctx_d = nc.dram_tensor("ctx", [BL, D], F32, kind="ExternalOutput")
    att_d = nc.dram_tensor("att", [BL, S], F32, kind="ExternalOutput")
    ncov_d = nc.dram_tensor("ncov", [BL, S], F32, kind="ExternalOutput")

    with tile.TileContext(nc) as tc:
        _emit(nc, tc, q_d, v_d, cov_d, w1_d, w2_d, w3_d, vv_d, b1_d, b2_d,
              b3_d, ctx_d, att_d, ncov_d)
    nc.compile()
    return nc


def _emit(nc, tc, q_d, v_d, cov_d, w1_d, w2_d, w3_d, vv_d, b1_d, b2_d, b3_d,
          ctx_d, att_d, ncov_d):
    from contextlib import ExitStack

    ctx = ExitStack()
    with ctx:
        const = ctx.enter_context(tc.tile_pool(name="const", bufs=1))
        # W1 as bf16, laid out [128(d within chunk), j*U + u]
        w1b = const.tile([P, DJ * U], BF16)
        for j in range(DJ):
            nc.gpsimd.dma_start(w1b[:, j * U:(j + 1) * U],
                                w1_d[j * P:(j + 1) * P, :])
        w3b = const.tile([1, U], BF16)
        nc.gpsimd.dma_start(w3b[:], w3_d.ap().unsqueeze(0))
        vwb = const.tile([P, UK], BF16)
        nc.gpsimd.dma_start(vwb[:], vv_d.ap().rearrange("(j p) -> p j", p=P))
        # bias sum b1+b2+b3 in [u%128, u//128] layout
        bsum = const.tile([P, UK], F32)
        btmp = const.tile([P, UK], F32)
        nc.sync.dma_start(bsum[:], b1_d.ap().rearrange("(j p) -> p j", p=P))
        nc.sync.dma_start(btmp[:], b2_d.ap().rearrange("(j p) -> p j", p=P))
        nc.vector.tensor_add(bsum[:], bsum[:], btmp[:])
        nc.sync.dma_start(btmp[:], b3_d.ap().rearrange("(j p) -> p j", p=P))
        nc.vector.tensor_add(bsum[:], bsum[:], btmp[:])

        # q_proj for all local batches: qp_sb[u%128, k*BL + b]
        qp_sb = const.tile([P, UK * BL], F32)
        with tc.tile_pool(name="wq", bufs=1) as wq_pool, \
             tc.tile_pool(name="qpp", bufs=2, space="PSUM") as qp_psum:
            qT = wq_pool.tile([P, DJ, BL], F32)
            for b in range(BL):
                nc.sync.dma_start(qT[:, :, b],
                                  q_d[b, :].rearrange("(j p) -> p j", p=P))
            w2sb = wq_pool.tile([P, DJ * U], F32)
            for j in range(DJ):
                nc.sync.dma_start(w2sb[:, j * U:(j + 1) * U],
                                  w2_d[j * P:(j + 1) * P, :])
            for k in range(UK):
                qp = qp_psum.tile([P, BL], F32)
                for j in range(DJ):
                    nc.tensor.matmul(
                        qp[:],
                        lhsT=w2sb[:, j * U + k * P: j * U + (k + 1) * P],
                        rhs=qT[:, j, :],
                        start=(j == 0), stop=(j == DJ - 1))
                # qp_sb = qp + bsum[:, k]  (fused psum->sbuf copy + bias add)
                nc.vector.tensor_scalar(
                    out=qp_sb[:, k * BL:(k + 1) * BL], in0=qp[:],
                    scalar1=bsum[:, k:k + 1], scalar2=None,
                    op0=mybir.AluOpType.add)

        vt_pool = ctx.enter_context(tc.tile_pool(name="vt", bufs=2))
        vb_pool = ctx.enter_context(tc.tile_pool(name="vb", bufs=4))
        vf2_pool = ctx.enter_context(tc.tile_pool(name="vf2", bufs=4))
        tt_pool = ctx.enter_context(tc.tile_pool(name="tt", bufs=3))
        row_pool = ctx.enter_context(tc.tile_pool(name="row", bufs=2))
        pt_psum = ctx.enter_context(tc.tile_pool(name="pt", bufs=3,
                                                 space="PSUM"))
        sc_psum = ctx.enter_context(tc.tile_pool(name="sc", bufs=2,
                                                 space="PSUM"))
        cx_psum = ctx.enter_context(tc.tile_pool(name="cx", bufs=2,
                                                 space="PSUM"))

        # Deferred per-batch state carried into the next batch section.
        prev = {}         # context-GEMM state of batch b-1
        pending_out = []  # output DMAs of batch b-1, emitted after T(b)

        def emit_context_loads(b):
            """bf16 re-read of values for batch b's context GEMM."""
            tiles = []
            for i in range(NST):
                vf2 = vf2_pool.tile([P, D], BF16)
                nc.gpsimd.dma_start(vf2[:], v_d[b, i * P:(i + 1) * P, :])
                tiles.append(vf2)
            return tiles

        def emit_context_mms(st):
            att_col = prev["att_col"]
            vf2 = prev["vf2"][st]
            for h in range(2):
                nc.tensor.matmul(
                    prev["cx"][h][:],
                    lhsT=att_col[:, st:st + 1],
                    rhs=vf2[:, h * SC:(h + 1) * SC],
                    start=(st == 0), stop=(st == NST - 1),
                    skip_group_check=True)

        def finish_context():
            b = prev["b"]
            ctx_row = row_pool.tile([1, D], F32, tag="ctxrow")
            for h in range(2):
                nc.vector.tensor_copy(ctx_row[:, h * SC:(h + 1) * SC],
                                      prev["cx"][h][:])
            nc.sync.dma_start(ctx_d[b:b + 1, :], ctx_row[:])
            prev.clear()

        for b in range(BL):
            # ---- P1: load values (bf16 casting DMA) + 3D xbar transpose ----
            covb = row_pool.tile([1, S], BF16, tag="covb")
            nc.gpsimd.dma_start(covb[:], cov_d[b:b + 1, :])
            covf = row_pool.tile([1, S], F32, tag="covf")
            nc.sync.dma_start(covf[:], cov_d[b:b + 1, :])

            vt = vt_pool.tile([P, DJ, S], BF16)
            for i in range(NST):
                vb = vb_pool.tile([P, D], BF16)
                nc.gpsimd.dma_start(vb[:], v_d[b, i * P:(i + 1) * P, :])
                nc.sync.dma_start_transpose(
                    out=vt[:, :, i * P:(i + 1) * P], in_=vb[:])

            # outputs of batch b-1 go out after this batch's transposes, so
            # the SP queue never blocks on softmax(b-1) ahead of T(b)
            for fn in pending_out:
                fn()
            pending_out.clear()

            # context loads of batch b-1 stream during this batch's GEMM1
            if prev:
                prev["vf2"] = emit_context_loads(prev["b"])
                prev["cx"] = [cx_psum.tile([1, SC], F32, name=f"cxp{h}",
                                           tag="cxp") for h in range(2)]

            # ---- P2: GEMM1 + tanh + score ----
            score_row = row_pool.tile([1, S], F32, tag="score")
            pending_score = None  # (k, tT, s_ps)
            slot = 0
            for sc in range(NSC):
                s_ps = sc_psum.tile([1, SC], F32)
                for k in range(UK):
                    pt = pt_psum.tile([P, SC], F32)
                    for j in range(DJ):
                        nc.tensor.matmul(
                            pt[:],
                            lhsT=w1b[:, j * U + k * P: j * U + (k + 1) * P],
                            rhs=vt[:, j, sc * SC:(sc + 1) * SC],
                            start=(j == 0), stop=False)
                    nc.tensor.matmul(
                        pt[:], lhsT=w3b[:, k * P:(k + 1) * P],
                        rhs=covb[:, sc * SC:(sc + 1) * SC],
                        start=False, stop=True)
                    # previous batch context mms, one s-tile per slot
                    if prev and 8 <= slot < 8 + NST:
                        emit_context_mms(slot - 8)
                        if slot == 8 + NST - 1:
                            finish_context()
                    # deferred score mm (one slot behind its tanh)
                    if pending_score is not None:
                        pk, ptT, ps = pending_score
                        nc.tensor.matmul(
                            ps[:], lhsT=vwb[:, pk:pk + 1], rhs=ptT[:],
                            start=(pk == 0), stop=(pk == UK - 1),
                            skip_group_check=True)
                    tT = tt_pool.tile([P, SC], BF16)
                    nc.scalar.activation(
                        tT[:], pt[:], TANH,
                        bias=qp_sb[:, k * BL + b: k * BL + b + 1])
                    pending_score = (k, tT, s_ps)
                    slot += 1
                # flush the last score mm of this s-chunk
                pk, ptT, ps = pending_score
                nc.tensor.matmul(ps[:], lhsT=vwb[:, pk:pk + 1], rhs=ptT[:],
                                 start=(pk == 0), stop=(pk == UK - 1),
                                 skip_group_check=True)
                pending_score = None
                nc.vector.tensor_copy(score_row[:, sc * SC:(sc + 1) * SC],
                                      s_ps[:])

            # ---- P3: softmax ----
            negmax = row_pool.tile([1, 1], F32, tag="negmax")
            nc.vector.reduce_max(negmax[:], score_row[:],
                                 axis=mybir.AxisListType.X, negate=True)
            zsum = row_pool.tile([1, 1], F32, tag="zsum")
            nc.scalar.activation(score_row[:], score_row[:], EXP,
                                 bias=negmax[:], accum_out=zsum[:])
            rz = row_pool.tile([1, 1], F32, tag="rz")
            nc.vector.reciprocal(rz[:], zsum[:])
            nc.vector.tensor_scalar(out=score_row[:], in0=score_row[:],
                                    scalar1=rz[:], scalar2=None,
                                    op0=mybir.AluOpType.mult)
            nc.vector.tensor_add(covf[:], covf[:], score_row[:])
            # att must land in DRAM before the att_col readback
            nc.sync.dma_start(att_d[b:b + 1, :], score_row[:])
            att_col = row_pool.tile([P, NST], BF16, tag="attcol")
            nc.gpsimd.dma_start(att_col[:],
                                att_d[b, :].rearrange("(t p) -> p t", p=P))
            prev.update({"b": b, "att_col": att_col})

            def make_out(b=b, covf=covf):
                def emit():
                    nc.sync.dma_start(ncov_d[b:b + 1, :], covf[:])
                return emit
            pending_out.append(make_out())

        # tail: context GEMM + outputs for the last batch
        for fn in pending_out:
            fn()
        prev["vf2"] = emit_context_loads(prev["b"])
        prev["cx"] = [cx_psum.tile([1, SC], F32, name=f"cxp{h}", tag="cxp")
                      for h in range(2)]
        for st in range(NST):
            emit_context_mms(st)
        finish_context()


def _get_nc():
    if "nc" not in _CACHE:
        _CACHE["nc"] = _build()
    return _CACHE["nc"]


def kernel(query, values, coverage, W1, b1, W2, b2, W3, b3, V, bV):
    query = np.ascontiguousarray(np.asarray(query, dtype=np.float32))
    values = np.ascontiguousarray(np.asarray(values, dtype=np.float32))
    coverage = np.ascontiguousarray(np.asarray(coverage, dtype=np.float32))
    shared = {
        "w1": np.ascontiguousarray(np.asarray(W1, np.float32)),
        "w2": np.ascontiguousarray(np.asarray(W2, np.float32)),
        "w3": np.ascontiguousarray(np.asarray(W3, np.float32).reshape(U)),
        "vv": np.ascontiguousarray(np.asarray(V, np.float32).reshape(U)),
        "b1": np.ascontiguousarray(np.asarray(b1, np.float32)),
        "b2": np.ascontiguousarray(np.asarray(b2, np.float32)),
        "b3": np.ascontiguousarray(np.asarray(b3, np.float32)),
    }
    in_maps = []
    for c in range(NCORES):
        lo, hi = c * BL, (c + 1) * BL
        in_maps.append({
            "q": query[lo:hi],
            "v": values[lo:hi],
            "cov": np.ascontiguousarray(coverage[lo:hi, :, 0]),
            **shared,
        })
    nc = _get_nc()
    res = run_bass_kernel_spmd(nc, in_maps, list(range(NCORES))).results
    context = np.concatenate([res[c]["ctx"] for c in range(NCORES)], axis=0)
    att = np.concatenate([res[c]["att"] for c in range(NCORES)], axis=0)
    ncov = np.concatenate([res[c]["ncov"] for c in range(NCORES)], axis=0)
    return (context,
            att.reshape(B, S, 1).astype(np.float32),
            ncov.reshape(B, S, 1).astype(np.float32))
